# revision 4
# baseline (speedup 1.0000x reference)
"""Trainium2 Bass kernel for nn_DepthwiseMultiKernelAttention.

Reference: dw = depthwise3x3(x, K); out = G_sm @ P @ A_sm @ dw.
Fold M = G_sm @ P @ A_sm; out[:, h, w] = sum_taps M9[t] @ x_shift(t),
M9[t][j, c] = M[j, c] * K[c, dy, dx].

v5 (80.7us modeled vs 86.8us previous best): per chunk of 16 output
rows (per core = 2 samples x 64 ch on 128 partitions, half image = 8
chunks of 8 two-row PSUM groups):
  PE:   2-3 taps bf16 + n fp8e4m3 DoubleRow matmuls covering n taps at
        x-quantization-only error -- each DR matmul's two k-tiles carry
        (value | delta-w correction) fp8 weight blocks against the same
        fp8 x stream, paired across taps so every tap gets w8 + dw8 =
        fp8(w) + fp8(w - fp8(w)) at half PE cost; + 1 mix matmul
        blockdiag(M.T) @ acc per group. The fp8 tap set is chosen at
        runtime (greedy, slab-estimated error, <= 3 taps under the
        2e-2 gate with margin; measured 1.70e-2 on the fixed seed).
  DVE:  scales s0,s2,s3 (tensor_scalar @4x, 16-row), l1b = s2 += s3
        (@2x), acc halves: s0 += l1b (in-place, 8-row), 1-row spill of
        Pool's add per half.
  Act:  scale s1 + 4-row PSUM->SBUF drains (fine pieces release PSUM
        banks early; the 16-row PSUM tile is exactly one chunk, so the
        next chunk's shifts depend on them).
  Pool: s0 += s1 (the slow 0.42-efficiency TT add; spans a chunk).
In-place adds keep the chain in 4 tile tags at bufs=3 so the list
scheduler has slack to hold the software-pipeline phase: all chunk-c
vector work runs during chunk c-1 (x loads run 2-3 chunks ahead).
In-order engine queues make emission order load-bearing: acc_h0(c)
before s0(c+1) before acc_h1(c) on DVE (mixes gate on acc, Pool gates
on s0); s1(c+1) after the first drain flush on Act. Tail chunks drain
4-row pieces on DVE+Act in parallel with 4-row stores.

Sharding (8 cores): core i = (sample pair i//2, row half i%2);
partitions hold (2 samples x 64 ch); host pre-pads the 1-pixel halo
and ships x as bf16 + fp8 streams; output returns as bf16.
"""

import numpy as np

B, C, H, W = 8, 64, 256, 256
N_CORES = 8
HH = H // 2
PR, PC = HH + 2, W + 2
RPC = 16  # rows per chunk
N_CHUNKS = HH // RPC
GV = 8

N_WARMUP = 16
DEFER = 1
STORE_DEFER = 2

LAST_EXEC_NS = None
_PROGRAMS = {}


def _off(t):
    return (t // 3) * PC + (t % 3)


def _dr_plan(fp8_taps):
    """Pack 2n (kind, tap, elem_off) k-tile entries into n DR matmuls,
    each pair ordered by offset so the rhs AP delta is positive. A lone
    tap (n=1) pairs its value and delta-w tiles with zero k-tiles at a
    one-element shift (side chosen to stay in bounds)."""
    ts = sorted(fp8_taps, key=_off)
    if len(ts) == 1:
        t = ts[0]
        off = _off(t)
        if t % 3 == 2 and t // 3 == 2:  # bottom-right tap: pad before
            return [
                (("z", t, off - 1), ("w", t, off)),
                (("z", t, off - 1), ("d", t, off)),
            ]
        return [
            (("w", t, off), ("z", t, off + 1)),
            (("d", t, off), ("z", t, off + 1)),
        ]
    entries = [("w", t, _off(t)) for t in ts] + [("d", t, _off(t)) for t in ts]
    plan = []
    for i in range(0, len(entries), 2):
        e0, e1 = entries[i], entries[i + 1]
        if e0[2] > e1[2]:
            e0, e1 = e1, e0
        assert e0[2] < e1[2]
        plan.append((e0, e1))
    return plan


def _build_program(pe_taps, fp8_taps, vec_taps, n_warmup=N_WARMUP,
                   defer=DEFER, store_defer=1,
                   spill=1, act_tail_drains=True, drain_rows=4, fillers=0,
                   c0new=False, s1_pos=0):
    import concourse.mybir as mybir
    import concourse.tile as tile
    from concourse import bacc
    import bass_rust

    f32 = mybir.dt.float32
    bf16 = mybir.dt.bfloat16
    fp8 = mybir.dt.float8e4
    add = mybir.AluOpType.add
    Copy = mybir.ActivationFunctionType.Copy

    plan = _dr_plan(fp8_taps)
    n_dr = len(plan)
    MIX = len(pe_taps)
    FILL0 = MIX + 1
    n_slots = FILL0 + 9

    nc = bacc.Bacc("TRN2", target_bir_lowering=False, debug=False,
                   num_devices=N_CORES)
    x_d = nc.dram_tensor("xs", [128, PR, PC], bf16, kind="ExternalInput").ap()
    x8_d = nc.dram_tensor("xs8", [128, PR, PC], fp8,
                          kind="ExternalInput").ap()
    w_d = nc.dram_tensor("wt", [128, n_slots, 128], bf16,
                         kind="ExternalInput").ap()
    w8_d = nc.dram_tensor("wt8", [128, max(n_dr, 1), 2, 128], fp8,
                          kind="ExternalInput").ap()
    k_d = nc.dram_tensor("kv", [128, 9], f32, kind="ExternalInput").ap()
    o_d = nc.dram_tensor("out", [128, HH, W], bf16, kind="ExternalOutput").ap()

    def dy_dx(si):
        return si // 3, si % 3

    def xs_of(xt, si, a, b):
        dy, dx = dy_dx(si)
        return xt[:, a + dy : b + dy, dx : dx + W]

    v0, v1, v2, v3 = vec_taps

    with tile.TileContext(nc) as tc:
        with (
            tc.tile_pool(name="wpool", bufs=1) as wpool,
            tc.tile_pool(name="xpool", bufs=3) as xpool,
            tc.tile_pool(name="ppool", bufs=1, space="PSUM") as ppool,
            tc.tile_pool(name="opool", bufs=3) as opool,
            tc.tile_pool(name="vpool", bufs=3) as vpool,
        ):
            ps_all = ppool.tile([128, 16, W], f32, name="ps", tag="ps")

            scratch = wpool.tile([128, 128], f32, tag="scratch")
            nc.vector.memset(scratch, 0.0)
            sc16 = scratch.bitcast(bf16)
            for wi in range(n_warmup):
                pw = 12 + 2 * (wi % 2)
                nc.tensor.matmul(
                    ps_all[:, pw, :128], lhsT=sc16[:, :128],
                    rhs=sc16[:, :128], start=True, stop=True,
                )

            wt = wpool.tile([128, n_slots, 128], bf16)
            nc.scalar.dma_start(out=wt, in_=w_d)
            wt8 = wpool.tile([128, max(n_dr, 1), 2, 128], fp8)
            nc.scalar.dma_start(out=wt8, in_=w8_d)
            kv = wpool.tile([128, 9], f32)
            nc.scalar.dma_start(out=kv, in_=k_d)

            def dr_rhs(xt8, wr, e0, e1):
                r0, c0 = e0[2] // PC, e0[2] % PC
                delta = e1[2] - e0[2]
                assert delta > 0
                base = xt8[:, wr + r0 : wr + r0 + 2, c0 : c0 + W]
                ap = base.copy()
                ap.ap = bass_rust.VecI64Pair(
                    [list(base.ap[0]), [delta, 2], [PC, 2], [1, W]]
                )
                return ap

            def load(c, split=False):
                xt = xpool.tile([128, RPC + 2, PC], bf16, name="xt", tag="xt",
                                bufs=4)
                xt8 = xpool.tile([128, RPC + 2, PC], fp8, name="xt8",
                                 tag="xt8", bufs=4)
                r0 = RPC * c
                if split and c0new:
                    # xt8 right after the first bf16 piece: fill groups can
                    # then use the cheap DR matmuls instead of 9 bf16 taps
                    nc.sync.dma_start(out=xt[:, :6, :], in_=x_d[:, :6, :])
                    nc.sync.dma_start(out=xt8, in_=x8_d[:, :18, :])
                    nc.sync.dma_start(out=xt[:, 6:12, :], in_=x_d[:, 6:12, :])
                    nc.sync.dma_start(out=xt[:, 12:, :], in_=x_d[:, 12:18, :])
                elif split:
                    nc.sync.dma_start(out=xt[:, :6, :], in_=x_d[:, :6, :])
                    nc.sync.dma_start(out=xt[:, 6:12, :], in_=x_d[:, 6:12, :])
                    nc.sync.dma_start(out=xt[:, 12:, :], in_=x_d[:, 12:18, :])
                    nc.sync.dma_start(out=xt8, in_=x8_d[:, :18, :])
                else:
                    nc.sync.dma_start(
                        out=xt, in_=x_d[:, r0 : r0 + RPC + 2, :]
                    )
                    nc.sync.dma_start(
                        out=xt8, in_=x8_d[:, r0 : r0 + RPC + 2, :]
                    )
                return xt, xt8

            def head_s0(xt, a=0):
                s0 = vpool.tile([128, 16, W], bf16, tag="s0")
                nc.vector.tensor_scalar_mul(
                    s0[:, a:16, :], xs_of(xt, v0, a, 16), kv[:, v0 : v0 + 1]
                )
                return s0

            def head_s1(xt, a=0):
                s1 = vpool.tile([128, 16, W], bf16, tag="s1")
                nc.scalar.activation(
                    out=s1[:, a:16, :], in_=xs_of(xt, v1, a, 16), func=Copy,
                    scale=kv[:, v1 : v1 + 1],
                )
                return s1

            def head_pool(s0, s1, halves=(0, 1)):
                """s0 += s1 on Pool (per 8-row half, optional DVE spill)."""
                for h in halves:
                    sp = spill if (spill < 10 or h == 0) else 0
                    if spill >= 10:
                        sp = spill - 10 if h == 0 else 0
                    a, b = 8 * h, 8 * h + 8 - sp
                    nc.gpsimd.tensor_add(
                        s0[:, a:b, :], s0[:, a:b, :], s1[:, a:b, :]
                    )
                    if sp:
                        nc.vector.tensor_add(
                            s0[:, b : 8 * h + 8, :],
                            s0[:, b : 8 * h + 8, :],
                            s1[:, b : 8 * h + 8, :],
                        )

            def head_l1b(xt, a=0):
                s2 = vpool.tile([128, 16, W], bf16, tag="s2")
                nc.vector.tensor_scalar_mul(
                    s2[:, a:16, :], xs_of(xt, v2, a, 16), kv[:, v2 : v2 + 1]
                )
                s3 = vpool.tile([128, 16, W], bf16, tag="s3")
                nc.vector.tensor_scalar_mul(
                    s3[:, a:16, :], xs_of(xt, v3, a, 16), kv[:, v3 : v3 + 1]
                )
                nc.vector.tensor_add(
                    s2[:, a:16, :], s2[:, a:16, :], s3[:, a:16, :]
                )
                return s2

            pending = []
            drained = []

            def flush_one():
                g, acc2, ot = pending.pop(0)
                pr = (2 * g) % 16
                if acc2 is not None:
                    nc.tensor.matmul(
                        ps_all[:, pr : pr + 2, :],
                        lhsT=wt[:, MIX, :], rhs=acc2,
                        start=False, stop=True,
                    )
                gpd = drain_rows // 2  # groups per drain piece
                if g % gpd == gpd - 1:
                    # drain the piece that just completed; finer pieces
                    # release PSUM banks earlier for the next chunk
                    cc = g // 8
                    pb = (2 * (g - gpd + 1)) % 16
                    rq = pb
                    tail = g // 4 >= 2 * N_CHUNKS - 2 and act_tail_drains
                    if tail:
                        for q in range(drain_rows // 4):
                            eng = (nc.vector.tensor_copy
                                   if (g // gpd + q) % 2 == 0
                                   else nc.scalar.copy)
                            eng(out=ot[:, rq + 4 * q : rq + 4 * q + 4, :],
                                in_=ps_all[:, pb + 4 * q : pb + 4 * q + 4, :])
                            drained.append((cc, ot, rq + 4 * q, 4))
                    else:
                        nc.scalar.copy(
                            out=ot[:, rq : rq + drain_rows, :],
                            in_=ps_all[:, pb : pb + drain_rows, :])

            stores = []
            xts = {0: load(0, split=True), 1: load(1)}
            # chunk 0: vector chain for h1 only
            a0 = 8 if (c0new or True) else 0
            s0s = {0: head_s0(xts[0][0], a=a0)}
            s1s = {0: head_s1(xts[0][0], a=a0)}
            l1bs = {0: head_l1b(xts[0][0], a=a0)}
            head_pool(s0s[0], s1s[0], halves=(1,))
            xts[2] = load(2)

            for c in range(N_CHUNKS):
                xt, xt8 = xts[c]
                while len(pending) > defer:
                    flush_one()
                if c + 3 < N_CHUNKS:
                    xts[c + 3] = load(c + 3)
                # DVE order matters (in-order queue): this chunk's acc_h0
                # first (it gates the first mixes), then the next chunk's
                # s0 (it gates Pool), then acc_h1, then the rest of the head
                acc = s0s.pop(c)
                l1b = l1bs.pop(c)

                def acc_half(h):
                    a, b2 = 8 * h, 8 * h + 8
                    nc.vector.tensor_add(
                        acc[:, a:b2, :], acc[:, a:b2, :], l1b[:, a:b2, :]
                    )

                if c > 0:
                    acc_half(0)
                tail_c = False
                if c + 1 < N_CHUNKS:
                    s0s[c + 1] = head_s0(xts[c + 1][0])
                    if s1_pos < 0:
                        s1s[c + 1] = head_s1(xts[c + 1][0])
                if not tail_c:
                    acc_half(1)
                if c + 1 < N_CHUNKS and s1_pos < 0:
                    head_pool(s0s[c + 1], s1s[c + 1])
                if c + 1 < N_CHUNKS:
                    l1bs[c + 1] = head_l1b(xts[c + 1][0])

                ot = opool.tile([128, RPC, W], bf16, tag="ot")
                if c > 0 and fillers:
                    # keep the PE p-state ramped across the chunk-boundary
                    # wait: cheap start+stop matmuls on banks already drained
                    for fi in range(fillers):
                        pw = 2 * (fi % 2)
                        nc.tensor.matmul(
                            ps_all[:, pw : pw + 1, :],
                            lhsT=sc16[:, :128], rhs=sc16[:, :256],
                            start=True, stop=True, skip_group_check=True,
                        )
                for gi in range(RPC // 2):
                    g = (RPC // 2) * c + gi
                    wr = 2 * gi
                    prr = (2 * g) % 16
                    pe_only = (c == 0 and wr < GV) or (tail_c and wr >= GV)
                    taps = list(enumerate(pe_taps))
                    use_dr = bool(plan) and (not pe_only or c0new)
                    if pe_only:
                        taps = taps + [(FILL0 + si, si) for si in vec_taps]
                        if not c0new:
                            taps = taps + [(FILL0 + si, si) for si in fp8_taps]
                    for k, (idx, si) in enumerate(taps):
                        dy, dx = dy_dx(si)
                        nc.tensor.matmul(
                            ps_all[:, prr : prr + 2, :],
                            lhsT=wt[:, idx, :],
                            rhs=xt[:, wr + dy : wr + dy + 2, dx : dx + W],
                            start=(k == 0),
                            stop=(pe_only and not use_dr
                                  and k == len(taps) - 1),
                        )
                    if use_dr:
                        for di, (e0, e1) in enumerate(plan):
                            nc.tensor.matmul(
                                ps_all[:, prr : prr + 2, :],
                                lhsT=wt8[:, di, :, :],
                                rhs=dr_rhs(xt8, wr, e0, e1),
                                start=False,
                                stop=(pe_only and di == len(plan) - 1),
                                perf_mode=mybir.MatmulPerfMode.DoubleRow,
                            )
                    acc2 = None
                    if not pe_only:
                        acc2 = acc[:, wr : wr + 2, :]
                    pending.append((g, acc2, ot))
                    while len(pending) > defer:
                        flush_one()
                    if gi == s1_pos and c + 1 < N_CHUNKS:
                        s1s[c + 1] = head_s1(xts[c + 1][0])
                        head_pool(s0s[c + 1], s1s[c + 1])

                stores.append((c, ot))
                if len(stores) > store_defer:
                    sc, sot = stores.pop(0)
                    nc.sync.dma_start(
                        out=o_d[:, RPC * sc : RPC * (sc + 1), :], in_=sot
                    )
            for sc, sot in stores:
                if sc == N_CHUNKS - 1 and act_tail_drains:
                    continue
                nc.sync.dma_start(
                    out=o_d[:, RPC * sc : RPC * (sc + 1), :], in_=sot
                )
            while pending:
                flush_one()
            for sc, sot, rq, n in drained:
                nc.sync.dma_start(
                    out=o_d[:, RPC * sc + rq : RPC * sc + rq + n, :],
                    in_=sot[:, rq : rq + n, :],
                )
    nc.compile()
    return nc


def _softmax_rows(a):
    a = a.astype(np.float64)
    a = np.exp(a - a.max(axis=1, keepdims=True))
    return a / a.sum(axis=1, keepdims=True)


def _fold(depthwise_weights, pointwise_weights, attention_weights,
          global_attention_weight):
    A = _softmax_rows(np.asarray(attention_weights))
    G = _softmax_rows(np.asarray(global_attention_weight))
    P = np.asarray(pointwise_weights)[:, :, 0, 0].astype(np.float64)
    M = G @ P @ A
    Kdw = np.asarray(depthwise_weights)[:, 0].astype(np.float64)  # (64,3,3)
    return M, Kdw


def _select_config(M, Kdw, x):
    """Pick fp8 taps (delta-w-corrected, so x-quantization error only)
    greedily while the estimated error stays under threshold; assign the
    rest: 3 (or 4) highest-energy taps to PE bf16, 4 to the vector chain."""
    import ml_dtypes
    import itertools

    xs = np.asarray(x[0], np.float32)
    slab = np.zeros((C, 34, W + 2), np.float32)
    slab[:, 1:33, 1:-1] = xs[:, :32, :]
    s8 = slab.astype(ml_dtypes.float8_e4m3).astype(np.float32)

    def w_t(si):
        return (M * Kdw[:, si // 3, si % 3][None, :]).astype(np.float32)

    diffs = {}
    outs = np.zeros((C, 32, W), np.float32)
    for t in range(9):
        dy, dx = t // 3, t % 3
        xv = slab[:, dy : dy + 32, dx : dx + W].reshape(C, -1)
        xv8 = s8[:, dy : dy + 32, dx : dx + W].reshape(C, -1)
        w = w_t(t)
        w8 = w.astype(ml_dtypes.float8_e4m3).astype(np.float32)
        dw8 = (w - w8).astype(ml_dtypes.float8_e4m3).astype(np.float32)
        exact = w @ xv
        diffs[t] = ((w8 + dw8) @ xv8 - exact).reshape(C, 32, W)
        outs += exact.reshape(C, 32, W)
    scale_est = np.abs(outs).max()

    # greedy: grow the fp8 set while estimated max error stays low
    TH = {1: 0.016, 2: 0.016, 3: 0.0175}
    chosen = []
    cur = np.zeros_like(diffs[0])
    for n in (1, 2, 3):
        cand = min((t for t in range(9) if t not in chosen),
                   key=lambda t: np.abs(cur + diffs[t]).max())
        e = np.abs(cur + diffs[cand]).max() / scale_est
        if e <= TH[n]:
            chosen.append(cand)
            cur = cur + diffs[cand]
        else:
            break
    rest = [t for t in range(9) if t not in chosen]
    energy = {t: float(np.linalg.norm(M * Kdw[:, t // 3, t % 3][None, :]))
              for t in rest}
    rest.sort(key=lambda t: -energy[t])
    n_pe = len(rest) - 4
    return tuple(rest[:n_pe]), tuple(chosen), tuple(rest[n_pe:])


def _make_weights(M, Kdw, pe_taps, fp8_taps, vec_taps):
    import ml_dtypes

    MIX = len(pe_taps)
    FILL0 = MIX + 1
    n_slots = FILL0 + 9
    wt = np.zeros((128, n_slots, 128), np.float32)

    def blk(si):
        return (M.T * Kdw[:, si // 3, si % 3][:, None]).astype(np.float32)

    for i, si in enumerate(pe_taps):
        b = blk(si)
        wt[:C, i, :C] = b
        wt[C:, i, C:] = b
    mixT = M.T.astype(np.float32)
    wt[:C, MIX, :C] = mixT
    wt[C:, MIX, C:] = mixT
    for si in range(9):
        b = blk(si)
        wt[:C, FILL0 + si, :C] = b
        wt[C:, FILL0 + si, C:] = b

    plan = _dr_plan(fp8_taps)
    n_dr = max(len(plan), 1)
    wt8 = np.zeros((128, n_dr, 2, 128), ml_dtypes.float8_e4m3)
    for di, pr in enumerate(plan):
        for ki, (kind, t, _o) in enumerate(pr):
            if kind == "z":
                continue
            b = blk(t)
            b8 = b.astype(ml_dtypes.float8_e4m3)
            if kind == "w":
                q = b8
            else:
                q = (b - b8.astype(np.float32)).astype(ml_dtypes.float8_e4m3)
            wt8[:C, di, ki, :C] = q
            wt8[C:, di, ki, C:] = q

    kva = np.empty((128, 9), np.float32)
    for si in range(9):
        kva[:C, si] = Kdw[:, si // 3, si % 3]
        kva[C:, si] = Kdw[:, si // 3, si % 3]
    return (wt.astype(ml_dtypes.bfloat16), wt8, kva)


def _make_shards(x):
    import ml_dtypes

    x = np.asarray(x, dtype=np.float32)
    shards = []
    for i in range(N_CORES):
        p, h = divmod(i, 2)
        buf = np.zeros((2, C, PR, PC), ml_dtypes.bfloat16)
        buf8 = np.zeros((2, C, PR, PC), ml_dtypes.float8_e4m3)
        r0 = HH * h - 1
        r1 = HH * h + HH + 1
        sr0, sr1 = max(r0, 0), min(r1, H)
        sl = x[2 * p : 2 * p + 2, :, sr0:sr1, :]
        buf[:, :, sr0 - r0 : sr1 - r0, 1 : 1 + W] = sl.astype(
            ml_dtypes.bfloat16)
        buf8[:, :, sr0 - r0 : sr1 - r0, 1 : 1 + W] = sl.astype(
            ml_dtypes.float8_e4m3)
        shards.append((buf.reshape(128, PR, PC), buf8.reshape(128, PR, PC)))
    return shards


def kernel(x, depthwise_weights, pointwise_weights, attention_weights,
           global_attention_weight):
    global LAST_EXEC_NS
    from concourse import bass_utils

    M, Kdw = _fold(depthwise_weights, pointwise_weights, attention_weights,
                   global_attention_weight)
    pe_taps, fp8_taps, vec_taps = _select_config(M, Kdw, np.asarray(x))
    key = (pe_taps, fp8_taps, vec_taps)
    if key not in _PROGRAMS:
        _PROGRAMS[key] = _build_program(pe_taps, fp8_taps, vec_taps)
    nc = _PROGRAMS[key]

    wt, wt8, kv = _make_weights(M, Kdw, pe_taps, fp8_taps, vec_taps)
    shards = _make_shards(x)
    in_maps = [
        {"xs": shards[i][0], "xs8": shards[i][1], "wt": wt, "wt8": wt8,
         "kv": kv}
        for i in range(N_CORES)
    ]
    res = bass_utils.run_bass_kernel_spmd(
        nc, in_maps, core_ids=list(range(N_CORES)), trace=False
    )
    LAST_EXEC_NS = res.exec_time_ns

    out = np.empty((B, C, H, W), np.float32)
    for i in range(N_CORES):
        p, h = divmod(i, 2)
        o = res.results[i]["out"].astype(np.float32).reshape(2, C, HH, W)
        out[2 * p : 2 * p + 2, :, HH * h : HH * h + HH, :] = o
    return out


# revision 5
# speedup vs baseline: 1.0138x; 1.0138x over previous
"""Trainium2 Bass kernel for nn_DepthwiseMultiKernelAttention.

Reference: dw = depthwise3x3(x, K); out = G_sm @ P @ A_sm @ dw.
Fold M = G_sm @ P @ A_sm; out[:, h, w] = sum_taps M9[t] @ x_shift(t),
M9[t][j, c] = M[j, c] * K[c, dy, dx].

v5 (80.7us modeled vs 86.8us previous best): per chunk of 16 output
rows (per core = 2 samples x 64 ch on 128 partitions, half image = 8
chunks of 8 two-row PSUM groups):
  PE:   2-3 taps bf16 + n fp8e4m3 DoubleRow matmuls covering n taps at
        x-quantization-only error -- each DR matmul's two k-tiles carry
        (value | delta-w correction) fp8 weight blocks against the same
        fp8 x stream, paired across taps so every tap gets w8 + dw8 =
        fp8(w) + fp8(w - fp8(w)) at half PE cost; + 1 mix matmul
        blockdiag(M.T) @ acc per group. The fp8 tap set is chosen at
        runtime (greedy, slab-estimated error, <= 3 taps under the
        2e-2 gate with margin; measured 1.70e-2 on the fixed seed).
  DVE:  scales s0,s2,s3 (tensor_scalar @4x, 16-row), l1b = s2 += s3
        (@2x), acc halves: s0 += l1b (in-place, 8-row), 1-row spill of
        Pool's add per half.
  Act:  scale s1 + 4-row PSUM->SBUF drains (fine pieces release PSUM
        banks early; the 16-row PSUM tile is exactly one chunk, so the
        next chunk's shifts depend on them).
  Pool: s0 += s1 (the slow 0.42-efficiency TT add; spans a chunk).
In-place adds keep the chain in 4 tile tags at bufs=3 so the list
scheduler has slack to hold the software-pipeline phase: all chunk-c
vector work runs during chunk c-1 (x loads run 2-3 chunks ahead).
In-order engine queues make emission order load-bearing: acc_h0(c)
before s0(c+1) before acc_h1(c) on DVE (mixes gate on acc, Pool gates
on s0); s1(c+1) after the first drain flush on Act. Tail chunks drain
4-row pieces on DVE+Act in parallel with 4-row stores.

Sharding (8 cores): core i = (sample pair i//2, row half i%2);
partitions hold (2 samples x 64 ch); host pre-pads the 1-pixel halo
and ships x as bf16 + fp8 streams; output returns as bf16.
"""

import numpy as np

B, C, H, W = 8, 64, 256, 256
N_CORES = 8
HH = H // 2
PR, PC = HH + 2, W + 2
RPC = 16  # rows per chunk
N_CHUNKS = HH // RPC
GV = 8

N_WARMUP = 16
DEFER = 1
STORE_DEFER = 2

LAST_EXEC_NS = None
_PROGRAMS = {}


def _off(t):
    return (t // 3) * PC + (t % 3)


def _dr_plan(fp8_taps):
    """Pack 2n (kind, tap, elem_off) k-tile entries into n DR matmuls,
    each pair ordered by offset so the rhs AP delta is positive. A lone
    tap (n=1) pairs its value and delta-w tiles with zero k-tiles at a
    one-element shift (side chosen to stay in bounds)."""
    ts = sorted(fp8_taps, key=_off)
    if len(ts) == 1:
        t = ts[0]
        off = _off(t)
        if t % 3 == 2 and t // 3 == 2:  # bottom-right tap: pad before
            return [
                (("z", t, off - 1), ("w", t, off)),
                (("z", t, off - 1), ("d", t, off)),
            ]
        return [
            (("w", t, off), ("z", t, off + 1)),
            (("d", t, off), ("z", t, off + 1)),
        ]
    entries = [("w", t, _off(t)) for t in ts] + [("d", t, _off(t)) for t in ts]
    plan = []
    for i in range(0, len(entries), 2):
        e0, e1 = entries[i], entries[i + 1]
        if e0[2] > e1[2]:
            e0, e1 = e1, e0
        assert e0[2] < e1[2]
        plan.append((e0, e1))
    return plan


def _build_program(pe_taps, fp8_taps, vec_taps, n_warmup=N_WARMUP,
                   defer=DEFER, store_defer=1,
                   spill=1, act_tail_drains=True, drain_rows=4, fillers=0,
                   c0new=False, s1_pos=0):
    import concourse.mybir as mybir
    import concourse.tile as tile
    from concourse import bacc
    import bass_rust

    f32 = mybir.dt.float32
    bf16 = mybir.dt.bfloat16
    fp8 = mybir.dt.float8e4
    add = mybir.AluOpType.add
    Copy = mybir.ActivationFunctionType.Copy

    plan = _dr_plan(fp8_taps)
    n_dr = len(plan)
    MIX = len(pe_taps)
    FILL0 = MIX + 1
    n_slots = FILL0 + 9

    nc = bacc.Bacc("TRN2", target_bir_lowering=False, debug=False,
                   num_devices=N_CORES)
    x_d = nc.dram_tensor("xs", [128, PR, PC], bf16, kind="ExternalInput").ap()
    x8_d = nc.dram_tensor("xs8", [128, PR, PC], fp8,
                          kind="ExternalInput").ap()
    w_d = nc.dram_tensor("wt", [128, n_slots, 128], bf16,
                         kind="ExternalInput").ap()
    w8_d = nc.dram_tensor("wt8", [128, max(n_dr, 1), 2, 128], fp8,
                          kind="ExternalInput").ap()
    k_d = nc.dram_tensor("kv", [128, 9], f32, kind="ExternalInput").ap()
    o_d = nc.dram_tensor("out", [128, HH, W], bf16, kind="ExternalOutput").ap()

    def dy_dx(si):
        return si // 3, si % 3

    def xs_of(xt, si, a, b):
        dy, dx = dy_dx(si)
        return xt[:, a + dy : b + dy, dx : dx + W]

    v0, v1, v2, v3 = vec_taps

    with tile.TileContext(nc) as tc:
        with (
            tc.tile_pool(name="wpool", bufs=1) as wpool,
            tc.tile_pool(name="xpool", bufs=3) as xpool,
            tc.tile_pool(name="ppool", bufs=1, space="PSUM") as ppool,
            tc.tile_pool(name="opool", bufs=3) as opool,
            tc.tile_pool(name="vpool", bufs=3) as vpool,
        ):
            ps_all = ppool.tile([128, 16, W], f32, name="ps", tag="ps")

            scratch = wpool.tile([128, 128], f32, tag="scratch")
            nc.vector.memset(scratch, 0.0)
            sc16 = scratch.bitcast(bf16)
            for wi in range(n_warmup):
                pw = 12 + 2 * (wi % 2)
                nc.tensor.matmul(
                    ps_all[:, pw, :128], lhsT=sc16[:, :128],
                    rhs=sc16[:, :128], start=True, stop=True,
                )

            wt = wpool.tile([128, n_slots, 128], bf16)
            nc.scalar.dma_start(out=wt, in_=w_d)
            wt8 = wpool.tile([128, max(n_dr, 1), 2, 128], fp8)
            nc.scalar.dma_start(out=wt8, in_=w8_d)
            kv = wpool.tile([128, 9], f32)
            nc.scalar.dma_start(out=kv, in_=k_d)

            def dr_rhs(xt8, wr, e0, e1):
                r0, c0 = e0[2] // PC, e0[2] % PC
                delta = e1[2] - e0[2]
                assert delta > 0
                base = xt8[:, wr + r0 : wr + r0 + 2, c0 : c0 + W]
                ap = base.copy()
                ap.ap = bass_rust.VecI64Pair(
                    [list(base.ap[0]), [delta, 2], [PC, 2], [1, W]]
                )
                return ap

            def load(c, split=False):
                xt = xpool.tile([128, RPC + 2, PC], bf16, name="xt", tag="xt",
                                bufs=4)
                xt8 = xpool.tile([128, RPC + 2, PC], fp8, name="xt8",
                                 tag="xt8", bufs=4)
                r0 = RPC * c
                if split and c0new:
                    # xt8 right after the first bf16 piece: fill groups can
                    # then use the cheap DR matmuls instead of 9 bf16 taps
                    nc.sync.dma_start(out=xt[:, :6, :], in_=x_d[:, :6, :])
                    nc.sync.dma_start(out=xt8, in_=x8_d[:, :18, :])
                    nc.sync.dma_start(out=xt[:, 6:12, :], in_=x_d[:, 6:12, :])
                    nc.sync.dma_start(out=xt[:, 12:, :], in_=x_d[:, 12:18, :])
                elif split:
                    nc.sync.dma_start(out=xt[:, :6, :], in_=x_d[:, :6, :])
                    nc.sync.dma_start(out=xt[:, 6:12, :], in_=x_d[:, 6:12, :])
                    nc.sync.dma_start(out=xt[:, 12:, :], in_=x_d[:, 12:18, :])
                    nc.sync.dma_start(out=xt8, in_=x8_d[:, :18, :])
                else:
                    nc.sync.dma_start(
                        out=xt, in_=x_d[:, r0 : r0 + RPC + 2, :]
                    )
                    nc.sync.dma_start(
                        out=xt8, in_=x8_d[:, r0 : r0 + RPC + 2, :]
                    )
                return xt, xt8

            def head_s0(xt, a=0):
                s0 = vpool.tile([128, 16, W], bf16, tag="s0")
                nc.vector.tensor_scalar_mul(
                    s0[:, a:16, :], xs_of(xt, v0, a, 16), kv[:, v0 : v0 + 1]
                )
                return s0

            def head_s1(xt, a=0):
                s1 = vpool.tile([128, 16, W], bf16, tag="s1")
                nc.scalar.activation(
                    out=s1[:, a:16, :], in_=xs_of(xt, v1, a, 16), func=Copy,
                    scale=kv[:, v1 : v1 + 1],
                )
                return s1

            def head_pool(s0, s1, halves=(0, 1)):
                """s0 += s1 on Pool (per 8-row half, optional DVE spill)."""
                for h in halves:
                    sp = spill if (spill < 10 or h == 0) else 0
                    if spill >= 10:
                        sp = spill - 10 if h == 0 else 0
                    a, b = 8 * h, 8 * h + 8 - sp
                    nc.gpsimd.tensor_add(
                        s0[:, a:b, :], s0[:, a:b, :], s1[:, a:b, :]
                    )
                    if sp:
                        nc.vector.tensor_add(
                            s0[:, b : 8 * h + 8, :],
                            s0[:, b : 8 * h + 8, :],
                            s1[:, b : 8 * h + 8, :],
                        )

            def head_l1b(xt, a=0):
                s2 = vpool.tile([128, 16, W], bf16, tag="s2")
                nc.vector.tensor_scalar_mul(
                    s2[:, a:16, :], xs_of(xt, v2, a, 16), kv[:, v2 : v2 + 1]
                )
                s3 = vpool.tile([128, 16, W], bf16, tag="s3")
                nc.vector.tensor_scalar_mul(
                    s3[:, a:16, :], xs_of(xt, v3, a, 16), kv[:, v3 : v3 + 1]
                )
                nc.vector.tensor_add(
                    s2[:, a:16, :], s2[:, a:16, :], s3[:, a:16, :]
                )
                return s2

            pending = []
            drained = []

            def flush_one():
                g, acc2, ot = pending.pop(0)
                pr = (2 * g) % 16
                if acc2 is not None:
                    nc.tensor.matmul(
                        ps_all[:, pr : pr + 2, :],
                        lhsT=wt[:, MIX, :], rhs=acc2,
                        start=False, stop=True,
                    )
                gpd = drain_rows // 2  # groups per drain piece
                if g % gpd == gpd - 1:
                    # drain the piece that just completed; finer pieces
                    # release PSUM banks earlier for the next chunk
                    cc = g // 8
                    pb = (2 * (g - gpd + 1)) % 16
                    rq = pb
                    tail = g // 4 >= 2 * N_CHUNKS - 2 and act_tail_drains
                    if tail:
                        for q in range(drain_rows // 4):
                            eng = (nc.vector.tensor_copy
                                   if (g // gpd + q) % 2 == 0
                                   else nc.scalar.copy)
                            eng(out=ot[:, rq + 4 * q : rq + 4 * q + 4, :],
                                in_=ps_all[:, pb + 4 * q : pb + 4 * q + 4, :])
                            drained.append((cc, ot, rq + 4 * q, 4))
                    else:
                        nc.scalar.copy(
                            out=ot[:, rq : rq + drain_rows, :],
                            in_=ps_all[:, pb : pb + drain_rows, :])

            stores = []
            xts = {0: load(0, split=True), 1: load(1)}
            # chunk 0: vector chain for h1 only
            a0 = 8 if (c0new or True) else 0
            s0s = {0: head_s0(xts[0][0], a=a0)}
            s1s = {0: head_s1(xts[0][0], a=a0)}
            l1bs = {0: head_l1b(xts[0][0], a=a0)}
            head_pool(s0s[0], s1s[0], halves=(1,))
            xts[2] = load(2)

            for c in range(N_CHUNKS):
                xt, xt8 = xts[c]
                while len(pending) > defer:
                    flush_one()
                if c + 3 < N_CHUNKS:
                    xts[c + 3] = load(c + 3)
                # DVE order matters (in-order queue): this chunk's acc_h0
                # first (it gates the first mixes), then the next chunk's
                # s0 (it gates Pool), then acc_h1, then the rest of the head
                acc = s0s.pop(c)
                l1b = l1bs.pop(c)

                def acc_half(h):
                    a, b2 = 8 * h, 8 * h + 8
                    nc.vector.tensor_add(
                        acc[:, a:b2, :], acc[:, a:b2, :], l1b[:, a:b2, :]
                    )

                if c > 0:
                    acc_half(0)
                tail_c = False
                if c + 1 < N_CHUNKS:
                    s0s[c + 1] = head_s0(xts[c + 1][0])
                    if s1_pos < 0:
                        s1s[c + 1] = head_s1(xts[c + 1][0])
                if c + 1 < N_CHUNKS and s1_pos < 0:
                    head_pool(s0s[c + 1], s1s[c + 1])
                if c + 1 < N_CHUNKS:
                    l1bs[c + 1] = head_l1b(xts[c + 1][0])
                if not tail_c:
                    acc_half(1)

                ot = opool.tile([128, RPC, W], bf16, tag="ot")
                if c > 0 and fillers:
                    # keep the PE p-state ramped across the chunk-boundary
                    # wait: cheap start+stop matmuls on banks already drained
                    for fi in range(fillers):
                        pw = 2 * (fi % 2)
                        nc.tensor.matmul(
                            ps_all[:, pw : pw + 1, :],
                            lhsT=sc16[:, :128], rhs=sc16[:, :256],
                            start=True, stop=True, skip_group_check=True,
                        )
                for gi in range(RPC // 2):
                    g = (RPC // 2) * c + gi
                    wr = 2 * gi
                    prr = (2 * g) % 16
                    pe_only = (c == 0 and wr < GV) or (tail_c and wr >= GV)
                    taps = list(enumerate(pe_taps))
                    use_dr = bool(plan) and (not pe_only or c0new)
                    if pe_only:
                        taps = taps + [(FILL0 + si, si) for si in vec_taps]
                        if not c0new:
                            taps = taps + [(FILL0 + si, si) for si in fp8_taps]
                    for k, (idx, si) in enumerate(taps):
                        dy, dx = dy_dx(si)
                        nc.tensor.matmul(
                            ps_all[:, prr : prr + 2, :],
                            lhsT=wt[:, idx, :],
                            rhs=xt[:, wr + dy : wr + dy + 2, dx : dx + W],
                            start=(k == 0),
                            stop=(pe_only and not use_dr
                                  and k == len(taps) - 1),
                        )
                    if use_dr:
                        for di, (e0, e1) in enumerate(plan):
                            nc.tensor.matmul(
                                ps_all[:, prr : prr + 2, :],
                                lhsT=wt8[:, di, :, :],
                                rhs=dr_rhs(xt8, wr, e0, e1),
                                start=False,
                                stop=(pe_only and di == len(plan) - 1),
                                perf_mode=mybir.MatmulPerfMode.DoubleRow,
                            )
                    acc2 = None
                    if not pe_only:
                        acc2 = acc[:, wr : wr + 2, :]
                    if fillers and c > 0 and gi == 1:
                        # in-order PE: these run exactly when the first
                        # mix would stall on acc_h0(c), keeping the
                        # p-state ramp hot; bank 2 is drained and not yet
                        # restarted at this point in the chunk
                        for fi in range(fillers):
                            nc.tensor.matmul(
                                ps_all[:, 4:5, :],
                                lhsT=sc16[:, :128], rhs=sc16[:, :256],
                                start=True, stop=True,
                                skip_group_check=True,
                            )
                    pending.append((g, acc2, ot))
                    while len(pending) > defer:
                        flush_one()
                    if gi == s1_pos and c + 1 < N_CHUNKS:
                        s1s[c + 1] = head_s1(xts[c + 1][0])
                        head_pool(s0s[c + 1], s1s[c + 1])

                stores.append((c, ot))
                if len(stores) > store_defer:
                    sc, sot = stores.pop(0)
                    nc.sync.dma_start(
                        out=o_d[:, RPC * sc : RPC * (sc + 1), :], in_=sot
                    )
            for sc, sot in stores:
                if sc == N_CHUNKS - 1 and act_tail_drains:
                    continue
                nc.sync.dma_start(
                    out=o_d[:, RPC * sc : RPC * (sc + 1), :], in_=sot
                )
            while pending:
                flush_one()
            for sc, sot, rq, n in drained:
                nc.sync.dma_start(
                    out=o_d[:, RPC * sc + rq : RPC * sc + rq + n, :],
                    in_=sot[:, rq : rq + n, :],
                )
    nc.compile()
    return nc


def _softmax_rows(a):
    a = a.astype(np.float64)
    a = np.exp(a - a.max(axis=1, keepdims=True))
    return a / a.sum(axis=1, keepdims=True)


def _fold(depthwise_weights, pointwise_weights, attention_weights,
          global_attention_weight):
    A = _softmax_rows(np.asarray(attention_weights))
    G = _softmax_rows(np.asarray(global_attention_weight))
    P = np.asarray(pointwise_weights)[:, :, 0, 0].astype(np.float64)
    M = G @ P @ A
    Kdw = np.asarray(depthwise_weights)[:, 0].astype(np.float64)  # (64,3,3)
    return M, Kdw


def _select_config(M, Kdw, x):
    """Pick fp8 taps (delta-w-corrected, so x-quantization error only)
    greedily while the estimated error stays under threshold; assign the
    rest: 3 (or 4) highest-energy taps to PE bf16, 4 to the vector chain."""
    import ml_dtypes
    import itertools

    xs = np.asarray(x[0], np.float32)
    slab = np.zeros((C, 34, W + 2), np.float32)
    slab[:, 1:33, 1:-1] = xs[:, :32, :]
    s8 = slab.astype(ml_dtypes.float8_e4m3).astype(np.float32)

    def w_t(si):
        return (M * Kdw[:, si // 3, si % 3][None, :]).astype(np.float32)

    diffs = {}
    outs = np.zeros((C, 32, W), np.float32)
    for t in range(9):
        dy, dx = t // 3, t % 3
        xv = slab[:, dy : dy + 32, dx : dx + W].reshape(C, -1)
        xv8 = s8[:, dy : dy + 32, dx : dx + W].reshape(C, -1)
        w = w_t(t)
        w8 = w.astype(ml_dtypes.float8_e4m3).astype(np.float32)
        dw8 = (w - w8).astype(ml_dtypes.float8_e4m3).astype(np.float32)
        exact = w @ xv
        diffs[t] = ((w8 + dw8) @ xv8 - exact).reshape(C, 32, W)
        outs += exact.reshape(C, 32, W)
    scale_est = np.abs(outs).max()

    # greedy: grow the fp8 set while estimated max error stays low
    TH = {1: 0.016, 2: 0.016, 3: 0.0175}
    chosen = []
    cur = np.zeros_like(diffs[0])
    for n in (1, 2, 3):
        cand = min((t for t in range(9) if t not in chosen),
                   key=lambda t: np.abs(cur + diffs[t]).max())
        e = np.abs(cur + diffs[cand]).max() / scale_est
        if e <= TH[n]:
            chosen.append(cand)
            cur = cur + diffs[cand]
        else:
            break
    rest = [t for t in range(9) if t not in chosen]
    energy = {t: float(np.linalg.norm(M * Kdw[:, t // 3, t % 3][None, :]))
              for t in rest}
    rest.sort(key=lambda t: -energy[t])
    n_pe = len(rest) - 4
    return tuple(rest[:n_pe]), tuple(chosen), tuple(rest[n_pe:])


def _make_weights(M, Kdw, pe_taps, fp8_taps, vec_taps):
    import ml_dtypes

    MIX = len(pe_taps)
    FILL0 = MIX + 1
    n_slots = FILL0 + 9
    wt = np.zeros((128, n_slots, 128), np.float32)

    def blk(si):
        return (M.T * Kdw[:, si // 3, si % 3][:, None]).astype(np.float32)

    for i, si in enumerate(pe_taps):
        b = blk(si)
        wt[:C, i, :C] = b
        wt[C:, i, C:] = b
    mixT = M.T.astype(np.float32)
    wt[:C, MIX, :C] = mixT
    wt[C:, MIX, C:] = mixT
    for si in range(9):
        b = blk(si)
        wt[:C, FILL0 + si, :C] = b
        wt[C:, FILL0 + si, C:] = b

    plan = _dr_plan(fp8_taps)
    n_dr = max(len(plan), 1)
    wt8 = np.zeros((128, n_dr, 2, 128), ml_dtypes.float8_e4m3)
    for di, pr in enumerate(plan):
        for ki, (kind, t, _o) in enumerate(pr):
            if kind == "z":
                continue
            b = blk(t)
            b8 = b.astype(ml_dtypes.float8_e4m3)
            if kind == "w":
                q = b8
            else:
                q = (b - b8.astype(np.float32)).astype(ml_dtypes.float8_e4m3)
            wt8[:C, di, ki, :C] = q
            wt8[C:, di, ki, C:] = q

    kva = np.empty((128, 9), np.float32)
    for si in range(9):
        kva[:C, si] = Kdw[:, si // 3, si % 3]
        kva[C:, si] = Kdw[:, si // 3, si % 3]
    return (wt.astype(ml_dtypes.bfloat16), wt8, kva)


def _make_shards(x):
    import ml_dtypes

    x = np.asarray(x, dtype=np.float32)
    shards = []
    for i in range(N_CORES):
        p, h = divmod(i, 2)
        buf = np.zeros((2, C, PR, PC), ml_dtypes.bfloat16)
        buf8 = np.zeros((2, C, PR, PC), ml_dtypes.float8_e4m3)
        r0 = HH * h - 1
        r1 = HH * h + HH + 1
        sr0, sr1 = max(r0, 0), min(r1, H)
        sl = x[2 * p : 2 * p + 2, :, sr0:sr1, :]
        buf[:, :, sr0 - r0 : sr1 - r0, 1 : 1 + W] = sl.astype(
            ml_dtypes.bfloat16)
        buf8[:, :, sr0 - r0 : sr1 - r0, 1 : 1 + W] = sl.astype(
            ml_dtypes.float8_e4m3)
        shards.append((buf.reshape(128, PR, PC), buf8.reshape(128, PR, PC)))
    return shards


def kernel(x, depthwise_weights, pointwise_weights, attention_weights,
           global_attention_weight):
    global LAST_EXEC_NS
    from concourse import bass_utils

    M, Kdw = _fold(depthwise_weights, pointwise_weights, attention_weights,
                   global_attention_weight)
    pe_taps, fp8_taps, vec_taps = _select_config(M, Kdw, np.asarray(x))
    key = (pe_taps, fp8_taps, vec_taps)
    if key not in _PROGRAMS:
        _PROGRAMS[key] = _build_program(pe_taps, fp8_taps, vec_taps)
    nc = _PROGRAMS[key]

    wt, wt8, kv = _make_weights(M, Kdw, pe_taps, fp8_taps, vec_taps)
    shards = _make_shards(x)
    in_maps = [
        {"xs": shards[i][0], "xs8": shards[i][1], "wt": wt, "wt8": wt8,
         "kv": kv}
        for i in range(N_CORES)
    ]
    res = bass_utils.run_bass_kernel_spmd(
        nc, in_maps, core_ids=list(range(N_CORES)), trace=False
    )
    LAST_EXEC_NS = res.exec_time_ns

    out = np.empty((B, C, H, W), np.float32)
    for i in range(N_CORES):
        p, h = divmod(i, 2)
        o = res.results[i]["out"].astype(np.float32).reshape(2, C, HH, W)
        out[2 * p : 2 * p + 2, :, HH * h : HH * h + HH, :] = o
    return out
